# revision 18
# baseline (speedup 1.0000x reference)
"""Trainium2 Bass kernel for nn_GateActivation (e3nn gate: 512x0e + 256x1o + 128x2e).

Strategy:
  - Data-parallel over rows: 65536 rows -> 8 cores x 8192 rows; weights replicated.
  - Host transposes each shard to feature-major [1920, R] with the l>0 irreps
    de-interleaved to plane-major order, so every on-chip matmul is a plain
    weights-stationary `out = W.T @ actsT` with no on-chip transposes at all.
  - fp16 end-to-end: x / weights / intermediates / output all float16 (HBM
    traffic halves vs fp32; matmul rate is identical to fp32r at 1 cyc/row,
    PSUM accumulation stays fp32). Tolerance is 2e-2; fp16 costs ~1e-3.
  - Gate activations use only the Sigmoid ACT table (silu = x * sigmoid(x) via
    a DVE multiply), so no activation-table swaps.
  - Engine budget per 512-row group (PE is critical at ~17us):
      PE   78 matmuls; ACT 7 sigmoids + 10 evacs; DVE 15 muls + 5 evacs;
      SP   3 input DMA issues; Pool(gpsimd) 2 output DMA issues (SWDGE) so
      neither SP nor ACT queues ever block on output completion.
"""

import os
import sys
from contextlib import ExitStack

import numpy as np

sys.path.insert(0, "/opt/trn_rl_repo")

import concourse.bass as bass  # noqa: E402
import concourse.tile as tile  # noqa: E402
from concourse import bacc, mybir  # noqa: E402
from concourse.bass_utils import run_bass_kernel_spmd  # noqa: E402

# Problem shape (hardcoded per harness contract)
N_ROWS = 65536
N_CORES = 8
R = N_ROWS // N_CORES  # rows per core
D_IN = 1920
M0, M1, M2 = 512, 256, 128
GRP = 512  # rows per on-chip group (matmul moving free dim)
NCHUNK = D_IN // 128  # 15 feature chunks of 128

F32 = mybir.dt.float32
SIGMOID = mybir.ActivationFunctionType.Sigmoid

# matmul/IO dtype: f16 (default) | bf16 | f32r
_DT_NAME = os.environ.get("BASS_GATE_DTYPE", "f16")
MDT = {"f16": mybir.dt.float16,
       "bf16": mybir.dt.bfloat16,
       "f32r": mybir.dt.float32r}[_DT_NAME]
NP_MDT = {"f16": np.float16, "bf16": None, "f32r": np.float32}[_DT_NAME]
ODT = F32 if _DT_NAME == "f32r" else MDT  # DRAM output dtype

# Output-DMA issuing engine: "gpsimd" (SWDGE, own queue) or "scalar" (HWDGE
# on ACT — in-order after the ACT evac copies, so its waits are pre-satisfied)
OUT_ENG = os.environ.get("BASS_GATE_OUT_ENGINE", "scalar")

# Stash of the last run's results for test harness introspection
last_results = None


def build_nc(rows=R, grp=GRP):
    """Build the per-core Bass program (SPMD; same program on all 8 cores)."""
    assert rows % grp == 0
    n_groups = rows // grp
    nc = bacc.Bacc("TRN2", target_bir_lowering=False, debug=False)

    xT = nc.dram_tensor("xT", [n_groups, 128, NCHUNK * grp], MDT, kind="ExternalInput")
    wall = nc.dram_tensor("wall", [128, 6912], MDT, kind="ExternalInput")
    outT = nc.dram_tensor("outT", [n_groups, 128, NCHUNK * grp], ODT,
                          kind="ExternalOutput")

    with TileKernel(nc) as tk:
        tk.emit(xT, wall, outT, n_groups, grp, rows)
    nc.compile()
    return nc


class TileKernel:
    def __init__(self, nc):
        self.nc = nc
        self.ctx = ExitStack()

    def __enter__(self):
        self.tc = self.ctx.enter_context(tile.TileContext(self.nc))
        return self

    def __exit__(self, *exc):
        return self.ctx.__exit__(*exc)

    def emit(self, xT, wall, outT, n_groups, grp, rows):
        nc, tc, ctx = self.nc, self.tc, self.ctx

        wpool = ctx.enter_context(tc.tile_pool(name="w", bufs=1))
        xpool = ctx.enter_context(tc.tile_pool(name="x", bufs=3))
        ypool = ctx.enter_context(tc.tile_pool(name="y", bufs=2))
        apool = ctx.enter_context(tc.tile_pool(name="act", bufs=24))
        pre_ps = ctx.enter_context(
            tc.tile_pool(name="pre_ps", bufs=4, space=bass.MemorySpace.PSUM))
        post_ps = ctx.enter_context(
            tc.tile_pool(name="post_ps", bufs=4, space=bass.MemorySpace.PSUM))

        # --- load weights resident for the whole kernel; pre-gate block
        # first so group 0's first matmul isn't gated on the full 1.8MB ---
        wt = wpool.tile([128, 6912], MDT, tag="wall")
        nc.sync.dma_start(wt[:, :4224], wall[:, :4224])

        def load_x(g):
            """Issue the 3 input DMAs for group g on SP; returns the tiles."""
            xt_s = xpool.tile([128, 4 * grp], MDT, tag="xt_s")
            nc.sync.dma_start(xt_s[:], xT[g, :, :4 * grp])
            xt_v1 = xpool.tile([128, 6 * grp], MDT, tag="xt_v1")
            nc.sync.dma_start(xt_v1[:], xT[g, :, 4 * grp:10 * grp])
            xt_v2 = xpool.tile([128, 5 * grp], MDT, tag="xt_v2")
            nc.sync.dma_start(xt_v2[:], xT[g, :, 10 * grp:])
            return xt_s, xt_v1, xt_v2

        def store_y(g, yts):
            """Issue the 4 output DMAs for group g on SP (waits are on the
            evac-completion semaphores, already satisfied when deferred)."""
            yt_a, yt_b = yts
            nc.sync.dma_start(outT[g, :, :4 * grp], yt_a[:, :4 * grp])
            nc.sync.dma_start(outT[g, :, 4 * grp:8 * grp], yt_a[:, 4 * grp:])
            nc.sync.dma_start(outT[g, :, 8 * grp:12 * grp], yt_b[:, :4 * grp])
            nc.sync.dma_start(outT[g, :, 12 * grp:], yt_b[:, 4 * grp:])

        xtiles = {0: load_x(0)}
        nc.sync.dma_start(wt[:, 4224:], wall[:, 4224:])
        if n_groups > 1:
            xtiles[1] = load_x(1)
        prev_y = None

        # PE warm-up: dummy matmuls on a zeroed tile keep the PE busy during
        # the initial weight/x DMA window (~4.4us), so the HAM clock-gate is
        # already at 2.4GHz when group 0's real matmuls start. Sized to end
        # just as the first weight+input DMAs land.
        warm_x = apool.tile([128, grp], MDT, tag="warm")
        nc.vector.memset(warm_x[:], 0)
        warm_ps = pre_ps.tile([128, grp], F32, tag="pre")
        for _ in range(12):
            nc.tensor.matmul(warm_ps[:], warm_x[:, :128], warm_x[:],
                             start=True, stop=True)

        def w0pre_s(k, m):
            return wt[:, k * 896 + m * 128:k * 896 + (m + 1) * 128]

        def w1pre_s(k, c):
            return wt[:, 3584 + k * 256 + c * 128:3584 + k * 256 + (c + 1) * 128]

        w2pre_s = wt[:, 4096:4224]

        def w0post_s(k, m):
            return wt[:, 4224 + k * 512 + m * 128:4224 + k * 512 + (m + 1) * 128]

        def w1post_s(k, c):
            return wt[:, 6272 + k * 256 + c * 128:6272 + k * 256 + (c + 1) * 128]

        w2post_s = wt[:, 6784:6912]

        for g in range(n_groups):
            # SP program order per group: [out g-1][in g+2] — both have
            # satisfied waits by now (two-ahead prefetch, xpool bufs=3), so
            # SP streams ~11.8us of DMA per group while PE computes ~16.6us.
            if prev_y is not None:
                store_y(g - 1, prev_y)
            if g + 2 < n_groups:
                xtiles[g + 2] = load_x(g + 2)
            xt_s, xt_v1, xt_v2 = xtiles.pop(g)

            def xtc(c):
                if c < 4:
                    return xt_s[:, c * grp:(c + 1) * grp]
                if c < 10:
                    return xt_v1[:, (c - 4) * grp:(c - 3) * grp]
                return xt_v2[:, (c - 10) * grp:(c - 9) * grp]

            # --- pre-gate scalar path: s_preT chunks m=0..6 ---
            # m in 0..3 -> silu chunks (sc), m in 4..6 -> gate chunks (sigmoid)
            # Emit gate chunks first so gates are ready when v-planes arrive.
            sc = [None] * 4
            gt = [None] * 3
            for m in (4, 5, 6, 0, 1, 2, 3):
                ps = pre_ps.tile([128, grp], F32, tag="pre")
                for k in range(4):
                    nc.tensor.matmul(
                        ps[:],
                        w0pre_s(k, m),
                        xtc(k),
                        start=(k == 0), stop=(k == 3))
                if m >= 4:
                    gch = apool.tile([128, grp], F32, tag="act")
                    nc.scalar.activation(gch[:], ps[:], SIGMOID)
                    gt[m - 4] = gch
                else:
                    sg = apool.tile([128, grp], F32, tag="act")
                    nc.scalar.activation(sg[:], ps[:], SIGMOID)
                    sch = apool.tile([128, grp], MDT, tag="act")
                    nc.vector.tensor_mul(sch[:], ps[:], sg[:])  # silu = x*sig(x)
                    sc[m] = sch

            # --- pre-gate v1 (3 planes x 2 v-chunks) + gating ---
            v1g = [[None] * 2 for _ in range(3)]
            for i in range(3):
                for c in range(2):
                    ps = pre_ps.tile([128, grp], F32, tag="pre")
                    for k in range(2):
                        nc.tensor.matmul(
                            ps[:],
                            w1pre_s(k, c),
                            xtc(4 + 2 * i + k),
                            start=(k == 0), stop=(k == 1))
                    vg = apool.tile([128, grp], MDT, tag="act")
                    nc.vector.tensor_mul(vg[:], ps[:], gt[c][:])
                    v1g[i][c] = vg

            # --- pre-gate v2 (5 planes) + gating ---
            v2g = [None] * 5
            for i in range(5):
                ps = pre_ps.tile([128, grp], F32, tag="pre")
                nc.tensor.matmul(
                    ps[:],
                    w2pre_s,
                    xtc(10 + i),
                    start=True, stop=True)
                vg = apool.tile([128, grp], MDT, tag="act")
                nc.vector.tensor_mul(vg[:], ps[:], gt[2][:])
                v2g[i] = vg

            # --- post-gate ---
            yt_a = ypool.tile([128, 8 * grp], ODT, tag="yt_a")
            yt_b = ypool.tile([128, 7 * grp], ODT, tag="yt_b")

            def ytc(chunk):
                if chunk < 8:
                    return yt_a[:, chunk * grp:(chunk + 1) * grp]
                return yt_b[:, (chunk - 8) * grp:(chunk - 7) * grp]

            def evac(ps, chunk, on_dve):
                if on_dve:
                    nc.vector.tensor_copy(ytc(chunk), ps[:])
                else:
                    nc.scalar.copy(ytc(chunk), ps[:])

            for m in range(4):  # scalar out chunks
                ps = post_ps.tile([128, grp], F32, tag="post")
                for k in range(4):
                    nc.tensor.matmul(
                        ps[:],
                        w0post_s(k, m),
                        sc[k][:],
                        start=(k == 0), stop=(k == 3))
                evac(ps, m, on_dve=False)

            for i in range(3):  # v1 out planes
                for c in range(2):
                    ps = post_ps.tile([128, grp], F32, tag="post")
                    for k in range(2):
                        nc.tensor.matmul(
                            ps[:],
                            w1post_s(k, c),
                            v1g[i][k][:],
                            start=(k == 0), stop=(k == 1))
                    evac(ps, 4 + 2 * i + c, on_dve=False)

            for i in range(5):  # v2 out planes
                ps = post_ps.tile([128, grp], F32, tag="post")
                nc.tensor.matmul(
                    ps[:],
                    w2post_s,
                    v2g[i][:],
                    start=True, stop=True)
                evac(ps, 10 + i, on_dve=True)

            prev_y = (yt_a, yt_b)

        # drain: the last group's outputs (waits resolve per-evac, so the
        # first chunks stream out while the last post matmuls still run)
        store_y(n_groups - 1, prev_y)


# ---------------------------------------------------------------------------
# Host-side layout transforms
# ---------------------------------------------------------------------------

def to_feature_major(xs):
    """[r, 1920] row-major -> [1920, r] feature-major, v1/v2 plane-major rows."""
    r = xs.shape[0]
    xT = np.empty((D_IN, r), np.float32)
    xT[:M0] = xs[:, :M0].T
    xT[M0:M0 + 3 * M1] = (
        xs[:, M0:M0 + 3 * M1].reshape(r, M1, 3).transpose(2, 1, 0).reshape(3 * M1, r))
    xT[M0 + 3 * M1:] = (
        xs[:, M0 + 3 * M1:].reshape(r, M2, 5).transpose(2, 1, 0).reshape(5 * M2, r))
    return xT


def from_feature_major(yT):
    """Inverse of to_feature_major."""
    r = yT.shape[1]
    out = np.empty((r, D_IN), np.float32)
    out[:, :M0] = yT[:M0].T
    out[:, M0:M0 + 3 * M1] = (
        yT[M0:M0 + 3 * M1].reshape(3, M1, r).transpose(2, 1, 0).reshape(r, 3 * M1))
    out[:, M0 + 3 * M1:] = (
        yT[M0 + 3 * M1:].reshape(5, M2, r).transpose(2, 1, 0).reshape(r, 5 * M2))
    return out


def _to_mdt(a):
    """Cast a float32 ndarray to the kernel's matmul dtype (host side)."""
    if NP_MDT is np.float32:
        return np.ascontiguousarray(a, dtype=np.float32)
    if NP_MDT is np.float16:
        return np.ascontiguousarray(a.astype(np.float16))
    # bf16: round-to-nearest-even via uint32 bit tricks, stored as the raw
    # uint16 view (run_bass_kernel_spmd matches on byte layout).
    u = np.ascontiguousarray(a, dtype=np.float32).view(np.uint32)
    rounded = ((u + 0x7FFF + ((u >> 16) & 1)) >> 16).astype(np.uint16)
    import ml_dtypes
    return rounded.view(ml_dtypes.bfloat16).reshape(a.shape)


def prep_weights(W0_pre, W1_pre, W2_pre, W0_post, W1_post, W2_post):
    f = np.float32

    def chunks(w, scale, kchunks):
        # [K, M] -> [128, kchunks*M]: partition p holds rows {k*128+p}
        return (w / np.sqrt(scale)).astype(f).reshape(
            kchunks, 128, -1).transpose(1, 0, 2).reshape(128, -1)

    wall = np.concatenate([
        chunks(W0_pre, M0, 4), chunks(W1_pre, M1, 2), chunks(W2_pre, M2, 1),
        chunks(W0_post, M0, 4), chunks(W1_post, M1, 2), chunks(W2_post, M2, 1),
    ], axis=1)
    assert wall.shape == (128, 6912), wall.shape
    return {"wall": _to_mdt(wall)}


def to_groups(xT, grp=GRP):
    """[1920, r] -> [G, 128, 15*grp] partition-major per-group blocks."""
    r = xT.shape[1]
    g = r // grp
    return _to_mdt(
        xT.reshape(NCHUNK, 128, g, grp).transpose(2, 1, 0, 3).reshape(
            g, 128, NCHUNK * grp))


def from_groups(xTg):
    """[G, 128, 15*grp] -> [1920, r] (float32)."""
    g = xTg.shape[0]
    grp = xTg.shape[2] // NCHUNK
    return np.ascontiguousarray(
        xTg.astype(np.float32).reshape(g, 128, NCHUNK, grp).transpose(
            2, 1, 0, 3)).reshape(D_IN, g * grp)


_nc_cache = {}


def _get_nc(rows=R):
    key = (rows, _DT_NAME, OUT_ENG)
    if key not in _nc_cache:
        _nc_cache[key] = build_nc(rows=rows)
    return _nc_cache[key]


def make_core_inputs(shard, wmaps):
    """Full per-core input map for a [rows, 1920] float32 shard."""
    m = {"xT": to_groups(to_feature_major(shard))}
    m.update(wmaps)
    return m


def decode_core_output(outT, rows):
    """[G, 128, 15*grp] device output -> [rows, 1920] float32."""
    return from_feature_major(from_groups(np.asarray(outT)))


def kernel(x, W0_pre, W1_pre, W2_pre, W0_post, W1_post, W2_post):
    global last_results
    x = np.asarray(x, dtype=np.float32)
    assert x.shape == (N_ROWS, D_IN), x.shape

    wmaps = prep_weights(
        np.asarray(W0_pre), np.asarray(W1_pre), np.asarray(W2_pre),
        np.asarray(W0_post), np.asarray(W1_post), np.asarray(W2_post))

    nc = _get_nc()
    in_maps = [make_core_inputs(x[c * R:(c + 1) * R], wmaps)
               for c in range(N_CORES)]

    trace = os.environ.get("BASS_GATE_TRACE", "0") == "1"
    last_results = run_bass_kernel_spmd(
        nc, in_maps, list(range(N_CORES)), trace=trace)

    out = np.empty((N_ROWS, D_IN), np.float32)
    for c in range(N_CORES):
        out[c * R:(c + 1) * R] = decode_core_output(
            last_results.results[c]["outT"], R)
    return out
